# revision 11
# baseline (speedup 1.0000x reference)
"""Trainium2 Bass kernel for nn_ElementRelationships.

Math: out[b,t,n,f] = input[b,t,n,f] * 1.1  if n < batch_set_size[b,t] else 0.

Pure data parallel over B (32) across 8 cores -> 4 batches/core.
Per core: x shard [4,64,128,256] f32 = 32 MiB in + 32 MiB out dense.

Layout: flatten bt = (b*64+t) in [0,256) per core.  Host sorts the 256
rows by set_size (descending) and permutes x accordingly (inverted when
reassembling the output).  The shard is viewed as [a=2, p=128, s=NSPLIT, m]
where device row r = a*128 + p holds sorted-rank-r's block, and tile (a,s)
covers n in [s*NPER, (s+1)*NPER).  After sorting, the rows that need
chunk s form a partition prefix [0, K[a][s]) — so each tile is a single
partition-prefix DMA, and fully-masked rows are neither loaded nor
stored (ExternalOutput buffers are donated pre-zeroed by
run_bass_via_pjrt, so skipped rows read back as zeros).

The ragged mask (with the 1.1 scale baked in) rides along as a [128,256]
f32 input tile; each data tile is multiplied in place on DVE by a
step-0-broadcast slice of it (per (row, n) scalar broadcast over f=256).
K is rounded up to a multiple of 8 and maxed across cores, which only
adds rows whose mask is all-zero in that chunk (stored as zeros —
still exact).

Loads issue on the SP HWDGE ring, stores on the ACT ring, so a store
stalled on its mul never head-blocks later loads.
"""

import numpy as np

from contextlib import ExitStack

import concourse.bass as bass
import concourse.tile as tile
from concourse import bacc, mybir
from concourse import bass_utils

B, T, N, F = 32, 64, 128, 256
SCALE = 1.1  # ALPHA + BETA
N_CORES = 8
BPC = B // N_CORES            # batches per core = 4
BT = BPC * T                  # 256 flattened (b,t) rows per core
A = BT // 128                 # 2 partition-groups of rows
NSPLIT = 8                    # n-chunks per group
NPER = N // NSPLIT            # n-values per chunk
M = NPER * F                  # free elems per partition per tile
BUFS = 8
KQUANT = 16                   # multiple-of-16 K -> every DMA splits evenly over all 16 SDMA engines

_CACHE = {}


NT = A * NSPLIT               # total tiles


def _bases(kpat):
    """SBUF base partition per tile. The SDMA engine split is positional
    per DMA (contiguous chunks of ceil(K/16) descriptors), so staggering
    bases does not help balance — keep every tile at partition 0 and get
    balance from K being a multiple of 16 instead."""
    return (0,) * NT


def _build(kpat):
    """Build + compile the SPMD program for prefix-count pattern `kpat`
    (tuple of NT ints in [0,128], multiples of KQUANT)."""
    bases = _bases(kpat)
    nc = bacc.Bacc(
        "TRN2",
        target_bir_lowering=False,
        debug=False,
        enable_asserts=False,
        num_devices=N_CORES,
    )
    x = nc.dram_tensor("x", [A, 128, NSPLIT, M], mybir.dt.float32,
                       kind="ExternalInput").ap()
    mask = nc.dram_tensor("mask", [128, NT * NPER], mybir.dt.float32,
                          kind="ExternalInput").ap()
    y = nc.dram_tensor("y", [A, 128, NSPLIT, M], mybir.dt.float32,
                       kind="ExternalOutput").ap()

    with tile.TileContext(nc) as tc:
        with ExitStack() as ctx:
            mask_pool = ctx.enter_context(tc.tile_pool(name="maskp", bufs=1))
            pool = ctx.enter_context(tc.tile_pool(name="xp", bufs=BUFS))

            mask_sb = mask_pool.tile([128, NT * NPER], mybir.dt.float32,
                                     name="mask_sb")
            nc.scalar.dma_start(mask_sb[:], mask[:])

            for a in range(A):
                for s in range(NSPLIT):
                    ti = a * NSPLIT + s
                    K = kpat[ti]
                    if K == 0:
                        continue
                    b = bases[ti]
                    t = pool.tile([128, M], mybir.dt.float32, name="xt")
                    nc.sync.dma_start(t[b:b + K, :], x[a, 0:K, s, :])
                    # compute windows are quadrant-restricted; run the mul
                    # over all 128 partitions (extra lanes are free on DVE,
                    # see mask==0, and are never stored).
                    t3 = t.rearrange("p (n f) -> p n f", f=F)
                    msl = mask_sb[:, ti * NPER:(ti + 1) * NPER]
                    nc.vector.tensor_mul(
                        t3, t3, msl.unsqueeze(2).broadcast_to((128, NPER, F))
                    )
                    nc.scalar.dma_start(y[a, 0:K, s, :], t[b:b + K, :])

    nc.compile()
    return nc


def _get_nc(kpat):
    if kpat not in _CACHE:
        _CACHE[kpat] = _build(kpat)
    return _CACHE[kpat]


def _host_prep(input_tensor, batch_set_size):
    """Sort rows per core, build in_maps and the global K pattern."""
    ss_all = np.asarray(batch_set_size).reshape(B, T).astype(np.int64)
    n_idx = np.arange(N, dtype=np.int64)

    perms = []
    sorted_ss = []
    xs_sorted = []
    kmat = np.zeros((N_CORES, NT), dtype=np.int64)
    for i in range(N_CORES):
        ss = ss_all[i * BPC:(i + 1) * BPC].reshape(BT)
        perm = np.argsort(-ss, kind="stable")
        perms.append(perm)
        ss_sorted = ss[perm]
        sorted_ss.append(ss_sorted)

        xs = np.asarray(input_tensor[i * BPC:(i + 1) * BPC],
                        dtype=np.float32).reshape(BT, N * F)
        xs_sorted.append(np.ascontiguousarray(xs[perm]))

        for a in range(A):
            g = ss_sorted[a * 128:(a + 1) * 128]
            for s in range(NSPLIT):
                kmat[i, a * NSPLIT + s] = int((g > s * NPER).sum())

    kmax = kmat.max(axis=0)
    kpat = tuple(
        int(min(128, -(-k // KQUANT) * KQUANT)) for k in kmax
    )
    bases = _bases(kpat)

    in_maps = []
    for i in range(N_CORES):
        x_dev = xs_sorted[i].reshape(A, 128, NSPLIT, M)
        mrows = (n_idx[None, :] < sorted_ss[i][:, None]).astype(np.float32)
        mrows *= np.float32(SCALE)                           # [BT, N] sorted
        mdev = np.zeros((128, NT * NPER), dtype=np.float32)
        for a in range(A):
            for s in range(NSPLIT):
                ti = a * NSPLIT + s
                K, b = kpat[ti], bases[ti]
                if K == 0:
                    continue
                mdev[b:b + K, ti * NPER:(ti + 1) * NPER] = \
                    mrows[a * 128:a * 128 + K, s * NPER:(s + 1) * NPER]
        in_maps.append({"x": x_dev, "mask": mdev})
    return in_maps, perms, kpat


def kernel(input_tensor, batch_set_size):
    input_tensor = np.asarray(input_tensor, dtype=np.float32)
    in_maps, perms, kpat = _host_prep(input_tensor, batch_set_size)

    if all(k == 0 for k in kpat):
        return np.zeros((B, T, N, F), dtype=np.float32)

    nc = _get_nc(kpat)
    res = bass_utils.run_bass_kernel_spmd(
        nc, in_maps, core_ids=list(range(N_CORES))
    )
    out = np.empty((B, T, N, F), dtype=np.float32)
    for i in range(N_CORES):
        y_rows = res.results[i]["y"].reshape(BT, N * F)
        dst = out[i * BPC:(i + 1) * BPC].reshape(BT, N * F)
        dst[perms[i]] = y_rows
    return out


# revision 13
# speedup vs baseline: 1.0328x; 1.0328x over previous
"""Trainium2 Bass kernel for nn_ElementRelationships.

Math: out[b,t,n,f] = input[b,t,n,f] * 1.1  if n < batch_set_size[b,t] else 0.

Pure data parallel over B (32) across 8 cores -> 4 batches/core.
Per core: x shard [4,64,128,256] f32 = 32 MiB in + 32 MiB out dense.

Layout: flatten bt = (b*64+t) in [0,256) per core.  Host sorts the 256
rows by set_size (descending) and permutes x accordingly (inverted when
reassembling the output).  The shard is viewed as [a=2, p=128, s=NSPLIT, m]
where device row r = a*128 + p holds sorted-rank-r's block, and tile (a,s)
covers n in [s*NPER, (s+1)*NPER).  After sorting, the rows that need
chunk s form a partition prefix [0, K[a][s]) — so each tile is a single
partition-prefix DMA, and fully-masked rows are neither loaded nor
stored (ExternalOutput buffers are donated pre-zeroed by
run_bass_via_pjrt, so skipped rows read back as zeros).

The ragged mask (with the 1.1 scale baked in) rides along as a [128,256]
f32 input tile; each data tile is multiplied in place on DVE by a
step-0-broadcast slice of it (per (row, n) scalar broadcast over f=256).
K is rounded up to a multiple of 8 and maxed across cores, which only
adds rows whose mask is all-zero in that chunk (stored as zeros —
still exact).

Loads issue on the SP HWDGE ring, stores on the ACT ring, so a store
stalled on its mul never head-blocks later loads.
"""

import numpy as np

from contextlib import ExitStack

import concourse.bass as bass
import concourse.tile as tile
from concourse import bacc, mybir
from concourse import bass_utils

B, T, N, F = 32, 64, 128, 256
SCALE = 1.1  # ALPHA + BETA
N_CORES = 8
BPC = B // N_CORES            # batches per core = 4
BT = BPC * T                  # 256 flattened (b,t) rows per core
A = BT // 128                 # 2 partition-groups of rows
NSPLIT = 16                   # n-chunks per group
NPER = N // NSPLIT            # n-values per chunk
M = NPER * F                  # free elems per partition per tile
BUFS = 12
KQUANT = 16                   # multiple-of-16 K -> every DMA splits evenly over all 16 SDMA engines

_CACHE = {}


NT = A * NSPLIT               # total tiles


def _bases(kpat):
    """SBUF base partition per tile. The SDMA engine split is positional
    per DMA (contiguous chunks of ceil(K/16) descriptors), so staggering
    bases does not help balance — keep every tile at partition 0 and get
    balance from K being a multiple of 16 instead."""
    return (0,) * NT


def _build(kpat):
    """Build + compile the SPMD program for prefix-count pattern `kpat`
    (tuple of NT ints in [0,128], multiples of KQUANT)."""
    bases = _bases(kpat)
    nc = bacc.Bacc(
        "TRN2",
        target_bir_lowering=False,
        debug=False,
        enable_asserts=False,
        num_devices=N_CORES,
    )
    x = nc.dram_tensor("x", [A, 128, NSPLIT, M], mybir.dt.float32,
                       kind="ExternalInput").ap()
    mask = nc.dram_tensor("mask", [128, NT * NPER], mybir.dt.float32,
                          kind="ExternalInput").ap()
    y = nc.dram_tensor("y", [A, 128, NSPLIT, M], mybir.dt.float32,
                       kind="ExternalOutput").ap()

    with tile.TileContext(nc) as tc:
        with ExitStack() as ctx:
            mask_pool = ctx.enter_context(tc.tile_pool(name="maskp", bufs=1))
            pool = ctx.enter_context(tc.tile_pool(name="xp", bufs=BUFS))

            mask_sb = mask_pool.tile([128, NT * NPER], mybir.dt.float32,
                                     name="mask_sb")
            nc.scalar.dma_start(mask_sb[:], mask[:])

            for a in range(A):
                for s in range(NSPLIT):
                    ti = a * NSPLIT + s
                    K = kpat[ti]
                    if K == 0:
                        continue
                    b = bases[ti]
                    t = pool.tile([128, M], mybir.dt.float32, name="xt")
                    nc.sync.dma_start(t[b:b + K, :], x[a, 0:K, s, :])
                    # compute windows are quadrant-restricted; run the mul
                    # over all 128 partitions (extra lanes are free, see
                    # mask==0, and are never stored). Spread muls over DVE
                    # and the otherwise-idle GpSimd (2x slower per op) so
                    # neither serial chain approaches the DMA span.
                    t3 = t.rearrange("p (n f) -> p n f", f=F)
                    msl = mask_sb[:, ti * NPER:(ti + 1) * NPER]
                    eng = nc.gpsimd if ti % 3 == 2 else nc.vector
                    eng.tensor_mul(
                        t3, t3, msl.unsqueeze(2).broadcast_to((128, NPER, F))
                    )
                    nc.scalar.dma_start(y[a, 0:K, s, :], t[b:b + K, :])

    nc.compile()
    return nc


def _get_nc(kpat):
    if kpat not in _CACHE:
        _CACHE[kpat] = _build(kpat)
    return _CACHE[kpat]


def _host_prep(input_tensor, batch_set_size):
    """Sort rows per core, build in_maps and the global K pattern."""
    ss_all = np.asarray(batch_set_size).reshape(B, T).astype(np.int64)
    n_idx = np.arange(N, dtype=np.int64)

    perms = []
    sorted_ss = []
    xs_sorted = []
    kmat = np.zeros((N_CORES, NT), dtype=np.int64)
    for i in range(N_CORES):
        ss = ss_all[i * BPC:(i + 1) * BPC].reshape(BT)
        perm = np.argsort(-ss, kind="stable")
        perms.append(perm)
        ss_sorted = ss[perm]
        sorted_ss.append(ss_sorted)

        xs = np.asarray(input_tensor[i * BPC:(i + 1) * BPC],
                        dtype=np.float32).reshape(BT, N * F)
        xs_sorted.append(np.ascontiguousarray(xs[perm]))

        for a in range(A):
            g = ss_sorted[a * 128:(a + 1) * 128]
            for s in range(NSPLIT):
                kmat[i, a * NSPLIT + s] = int((g > s * NPER).sum())

    kmax = kmat.max(axis=0)
    kpat = tuple(
        int(min(128, -(-k // KQUANT) * KQUANT)) for k in kmax
    )
    bases = _bases(kpat)

    in_maps = []
    for i in range(N_CORES):
        x_dev = xs_sorted[i].reshape(A, 128, NSPLIT, M)
        mrows = (n_idx[None, :] < sorted_ss[i][:, None]).astype(np.float32)
        mrows *= np.float32(SCALE)                           # [BT, N] sorted
        mdev = np.zeros((128, NT * NPER), dtype=np.float32)
        for a in range(A):
            for s in range(NSPLIT):
                ti = a * NSPLIT + s
                K, b = kpat[ti], bases[ti]
                if K == 0:
                    continue
                mdev[b:b + K, ti * NPER:(ti + 1) * NPER] = \
                    mrows[a * 128:a * 128 + K, s * NPER:(s + 1) * NPER]
        in_maps.append({"x": x_dev, "mask": mdev})
    return in_maps, perms, kpat


def kernel(input_tensor, batch_set_size):
    input_tensor = np.asarray(input_tensor, dtype=np.float32)
    in_maps, perms, kpat = _host_prep(input_tensor, batch_set_size)

    if all(k == 0 for k in kpat):
        return np.zeros((B, T, N, F), dtype=np.float32)

    nc = _get_nc(kpat)
    res = bass_utils.run_bass_kernel_spmd(
        nc, in_maps, core_ids=list(range(N_CORES))
    )
    out = np.empty((B, T, N, F), dtype=np.float32)
    for i in range(N_CORES):
        y_rows = res.results[i]["y"].reshape(BT, N * F)
        dst = out[i * BPC:(i + 1) * BPC].reshape(BT, N * F)
        dst[perms[i]] = y_rows
    return out


# revision 14
# speedup vs baseline: 1.1217x; 1.0862x over previous
"""Trainium2 Bass kernel for nn_ElementRelationships.

Math: out[b,t,n,f] = input[b,t,n,f] * 1.1  if n < batch_set_size[b,t] else 0.

Pure data parallel over B (32) across 8 cores -> 4 batches/core; per core
the shard is [4,64,128,256] f32 = 32 MiB in / 32 MiB out dense.

Only cells with n < set_size contribute (the rest of the output is
zero), so the host packs exactly the ACTIVE f-rows (an f-row = 256
contiguous floats for one (b,t,n)) into a contiguous stream, cut into C
uniform [128, W] tiles. Every DMA is therefore a full-width, fully
contiguous, dense transfer — max SDMA efficiency — and the device
program depends only on C, not on the set_size values.

To keep one SPMD program across the 8 cores, rows are sorted by
set_size (descending) per core and the per-rank envelope
sspat[r] = max_i sorted_ss_i[r] defines a shared stream layout; each
core's mask (a [128, C*W/F] f32 tile, one value per f-row, with the
1.1 scale baked in) zeroes its own inactive/padded cells, so the
result is exact. The host scatters the device stream back into the
zero-initialised dense output (device writes every packed byte; no
reliance on output buffer pre-zeroing).

Loads issue on the SP HWDGE ring, stores on the ACT ring (a store
stalled on its mul must not head-block loads); the per-tile mask
multiply alternates between DVE and the otherwise-idle GpSimd so
neither engine's serial chain approaches the DMA span.
"""

import numpy as np

from contextlib import ExitStack

import concourse.bass as bass
import concourse.tile as tile
from concourse import bacc, mybir
from concourse import bass_utils

B, T, N, F = 32, 64, 128, 256
SCALE = 1.1  # ALPHA + BETA
N_CORES = 8
BPC = B // N_CORES            # batches per core = 4
BT = BPC * T                  # 256 flattened (b,t) rows per core
W = 2048                      # stream elems per partition per tile (8 KiB)
WF = W // F                   # f-rows per partition per tile
TILE_ROWS = 128 * WF          # f-rows per tile
BUFS = 12

_CACHE = {}


def _build(C):
    """Build + compile the SPMD program for C stream tiles."""
    nc = bacc.Bacc(
        "TRN2",
        target_bir_lowering=False,
        debug=False,
        enable_asserts=False,
        num_devices=N_CORES,
    )
    x = nc.dram_tensor("x", [C, 128, W], mybir.dt.float32,
                       kind="ExternalInput").ap()
    mask = nc.dram_tensor("mask", [128, C * WF], mybir.dt.float32,
                          kind="ExternalInput").ap()
    y = nc.dram_tensor("y", [C, 128, W], mybir.dt.float32,
                       kind="ExternalOutput").ap()

    with tile.TileContext(nc) as tc:
        with ExitStack() as ctx:
            mask_pool = ctx.enter_context(tc.tile_pool(name="maskp", bufs=1))
            pool = ctx.enter_context(tc.tile_pool(name="xp", bufs=BUFS))

            mask_sb = mask_pool.tile([128, C * WF], mybir.dt.float32,
                                     name="mask_sb")
            nc.scalar.dma_start(mask_sb[:], mask[:])

            for c in range(C):
                t = pool.tile([128, W], mybir.dt.float32, name="xt")
                nc.sync.dma_start(t[:], x[c])
                t3 = t.rearrange("p (j f) -> p j f", f=F)
                msl = mask_sb[:, c * WF:(c + 1) * WF]
                eng = nc.gpsimd if c % 3 == 2 else nc.vector
                eng.tensor_mul(
                    t3, t3, msl.unsqueeze(2).broadcast_to((128, WF, F))
                )
                nc.scalar.dma_start(y[c], t[:])

    nc.compile()
    return nc


def _get_nc(C):
    if C not in _CACHE:
        _CACHE[C] = _build(C)
    return _CACHE[C]


def _host_prep(input_tensor, batch_set_size):
    """Sort rows per core, build the shared stream layout and in_maps."""
    ss_all = np.asarray(batch_set_size).reshape(B, T).astype(np.int64)

    perms = []
    sorted_ss = np.empty((N_CORES, BT), dtype=np.int64)
    for i in range(N_CORES):
        ss = ss_all[i * BPC:(i + 1) * BPC].reshape(BT)
        perm = np.argsort(-ss, kind="stable")
        perms.append(perm)
        sorted_ss[i] = ss[perm]

    sspat = sorted_ss.max(axis=0)                      # envelope, sorted desc
    n_active = int(sspat.sum())                        # active f-rows / core
    if n_active == 0:
        return None, perms, 0, None, 0

    C = -(-n_active // TILE_ROWS)
    n_pad = C * TILE_ROWS

    # stream f-row q -> (rank r_arr[q], n = n_arr[q]); same for all cores
    r_arr = np.repeat(np.arange(BT), sspat)
    n_arr = np.concatenate([np.arange(k) for k in sspat]) if n_active else \
        np.zeros(0, np.int64)
    row_idx = r_arr * N + n_arr                        # into [BT*N] f-rows

    in_maps = []
    for i in range(N_CORES):
        xs = np.asarray(input_tensor[i * BPC:(i + 1) * BPC],
                        dtype=np.float32).reshape(BT, N, F)
        xs_sorted_rows = xs[perms[i]].reshape(BT * N, F)
        x_dev = np.zeros((n_pad, F), dtype=np.float32)
        x_dev[:n_active] = xs_sorted_rows[row_idx]
        x_dev = x_dev.reshape(C, 128, W)

        m = np.zeros(n_pad, dtype=np.float32)
        m[:n_active] = SCALE * (n_arr < sorted_ss[i][r_arr])
        mdev = np.ascontiguousarray(
            m.reshape(C, 128, WF).transpose(1, 0, 2).reshape(128, C * WF)
        )
        in_maps.append({"x": x_dev, "mask": mdev})
    return in_maps, perms, C, row_idx, n_active


def kernel(input_tensor, batch_set_size):
    input_tensor = np.asarray(input_tensor, dtype=np.float32)
    in_maps, perms, C, row_idx, n_active = _host_prep(
        input_tensor, batch_set_size)

    out = np.zeros((B, T, N, F), dtype=np.float32)
    if C == 0:
        return out

    nc = _get_nc(C)
    res = bass_utils.run_bass_kernel_spmd(
        nc, in_maps, core_ids=list(range(N_CORES))
    )
    for i in range(N_CORES):
        y_rows = res.results[i]["y"].reshape(-1, F)[:n_active]
        o_sorted = np.zeros((BT * N, F), dtype=np.float32)
        o_sorted[row_idx] = y_rows
        dst = out[i * BPC:(i + 1) * BPC].reshape(BT, N, F)
        dst[perms[i]] = o_sorted.reshape(BT, N, F)
    return out


# revision 18
# speedup vs baseline: 1.1848x; 1.0562x over previous
"""Trainium2 Bass kernel for nn_ElementRelationships.

Math: out[b,t,n,f] = input[b,t,n,f] * 1.1  if n < batch_set_size[b,t] else 0.

Pure data parallel over B (32) across 8 cores -> 4 batches/core; per core
the shard is [4,64,128,256] f32 = 32 MiB in / 32 MiB out dense.

Only cells with n < set_size contribute (the rest of the output is
zero), so the host packs exactly the ACTIVE f-rows (an f-row = 256
contiguous floats for one (b,t,n)) into a contiguous stream, cut into C
uniform [128, W] tiles. Every DMA is therefore a full-width, fully
contiguous, dense transfer — max SDMA efficiency — and the device
program depends only on C, not on the set_size values.

To keep one SPMD program across the 8 cores, rows are sorted by
set_size (descending) per core and the per-rank envelope
sspat[r] = max_i sorted_ss_i[r] defines a shared stream layout; each
core's mask (a [128, C*W/F] f32 tile, one value per f-row, with the
1.1 scale baked in) zeroes its own inactive/padded cells, so the
result is exact. The host scatters the device stream back into the
zero-initialised dense output (device writes every packed byte; no
reliance on output buffer pre-zeroing).

Loads issue on the SP HWDGE ring, stores on the ACT ring (a store
stalled on its mul must not head-block loads); the per-tile mask
multiply alternates between DVE and the otherwise-idle GpSimd so
neither engine's serial chain approaches the DMA span.
"""

import numpy as np

from contextlib import ExitStack

import concourse.bass as bass
import concourse.tile as tile
from concourse import bacc, mybir
from concourse import bass_utils

B, T, N, F = 32, 64, 128, 256
SCALE = 1.1  # ALPHA + BETA
N_CORES = 8
BPC = B // N_CORES            # batches per core = 4
BT = BPC * T                  # 256 flattened (b,t) rows per core
W = 4096                      # stream elems per partition per tile (16 KiB)
WF = W // F                   # f-rows per partition per tile
TILE_ROWS = 128 * WF          # f-rows per tile
BUFS = 6

_CACHE = {}


def _build(C, W2):
    """Build + compile the SPMD program: C full [128, W] stream tiles plus
    (if W2 > 0) one narrowed [128, W2] tail tile."""
    nc = bacc.Bacc(
        "TRN2",
        target_bir_lowering=False,
        debug=False,
        enable_asserts=False,
        num_devices=N_CORES,
    )
    widths = [W] * C + ([W2] if W2 else [])
    wf_tot = sum(w // F for w in widths)
    total = sum(widths)
    # 1-D stream; each tile is a contiguous [128, w] block (partition-major)
    x = nc.dram_tensor("x", [128 * total], mybir.dt.float32,
                       kind="ExternalInput").ap()
    mask = nc.dram_tensor("mask", [128, wf_tot], mybir.dt.float32,
                          kind="ExternalInput").ap()
    y = nc.dram_tensor("y", [128 * total], mybir.dt.float32,
                       kind="ExternalOutput").ap()

    with tile.TileContext(nc) as tc:
        with ExitStack() as ctx:
            mask_pool = ctx.enter_context(tc.tile_pool(name="maskp", bufs=1))
            pool = ctx.enter_context(tc.tile_pool(name="xp", bufs=BUFS))

            mask_sb = mask_pool.tile([128, wf_tot], mybir.dt.float32,
                                     name="mask_sb")
            nc.scalar.dma_start(mask_sb[:], mask[:])

            off = joff = 0
            for w in widths:
                wf = w // F
                xv = x[off:off + 128 * w].rearrange("(p m) -> p m", p=128)
                yv = y[off:off + 128 * w].rearrange("(p m) -> p m", p=128)
                t = pool.tile([128, W], mybir.dt.float32, name="xt")
                nc.sync.dma_start(t[:, 0:w], xv)
                t3 = t[:, 0:w].rearrange("p (j f) -> p j f", f=F)
                msl = mask_sb[:, joff:joff + wf]
                nc.vector.tensor_mul(
                    t3, t3, msl.unsqueeze(2).broadcast_to((128, wf, F))
                )
                nc.scalar.dma_start(yv, t[:, 0:w])
                off += 128 * w
                joff += wf

    nc.compile()
    return nc


def _get_nc(C, W2):
    if (C, W2) not in _CACHE:
        _CACHE[(C, W2)] = _build(C, W2)
    return _CACHE[(C, W2)]


def _host_prep(input_tensor, batch_set_size):
    """Sort rows per core, build the shared stream layout and in_maps."""
    ss_all = np.asarray(batch_set_size).reshape(B, T).astype(np.int64)

    perms = []
    sorted_ss = np.empty((N_CORES, BT), dtype=np.int64)
    for i in range(N_CORES):
        ss = ss_all[i * BPC:(i + 1) * BPC].reshape(BT)
        perm = np.argsort(-ss, kind="stable")
        perms.append(perm)
        sorted_ss[i] = ss[perm]

    sspat = sorted_ss.max(axis=0)                      # envelope, sorted desc
    n_active = int(sspat.sum())                        # active f-rows / core
    if n_active == 0:
        return None, perms, (0, 0), None, 0

    C = n_active // TILE_ROWS
    rem = n_active - C * TILE_ROWS                     # f-rows in tail tile
    W2 = -(-rem // 128) * F if rem else 0              # tail width (elems)
    n_pad = C * TILE_ROWS + 128 * (W2 // F)
    widths = [W] * C + ([W2] if W2 else [])
    wfs = [w // F for w in widths]
    wf_tot = sum(wfs)

    # stream f-row q -> (rank r_arr[q], n = n_arr[q]); same for all cores
    r_arr = np.repeat(np.arange(BT), sspat)
    n_arr = np.concatenate([np.arange(k) for k in sspat if k])
    row_idx = r_arr * N + n_arr                        # into [BT*N] f-rows

    in_maps = []
    for i in range(N_CORES):
        xs = np.asarray(input_tensor[i * BPC:(i + 1) * BPC],
                        dtype=np.float32).reshape(BT, N, F)
        xs_sorted_rows = xs[perms[i]].reshape(BT * N, F)
        x_dev = np.zeros((n_pad, F), dtype=np.float32)
        x_dev[:n_active] = xs_sorted_rows[row_idx]

        m = np.zeros(n_pad, dtype=np.float32)
        m[:n_active] = SCALE * (n_arr < sorted_ss[i][r_arr])
        mdev = np.zeros((128, wf_tot), dtype=np.float32)
        ro = joff = 0
        for wf in wfs:
            mdev[:, joff:joff + wf] = m[ro:ro + 128 * wf].reshape(128, wf)
            ro += 128 * wf
            joff += wf
        in_maps.append({"x": x_dev.reshape(-1), "mask": mdev})
    return in_maps, perms, (C, W2), row_idx, n_active


def kernel(input_tensor, batch_set_size):
    input_tensor = np.asarray(input_tensor, dtype=np.float32)
    in_maps, perms, (C, W2), row_idx, n_active = _host_prep(
        input_tensor, batch_set_size)

    out = np.zeros((B, T, N, F), dtype=np.float32)
    if n_active == 0:
        return out

    nc = _get_nc(C, W2)
    res = bass_utils.run_bass_kernel_spmd(
        nc, in_maps, core_ids=list(range(N_CORES))
    )
    for i in range(N_CORES):
        y_rows = res.results[i]["y"].reshape(-1, F)[:n_active]
        o_sorted = np.zeros((BT * N, F), dtype=np.float32)
        o_sorted[row_idx] = y_rows
        dst = out[i * BPC:(i + 1) * BPC].reshape(BT, N, F)
        dst[perms[i]] = o_sorted.reshape(BT, N, F)
    return out


# revision 20
# speedup vs baseline: 1.3864x; 1.1702x over previous
"""Trainium2 Bass kernel for nn_ElementRelationships.

Math: out[b,t,n,f] = input[b,t,n,f] * 1.1  if n < batch_set_size[b,t] else 0.

Pure data parallel over B (32) across 8 cores -> 4 batches/core; per core
the shard is [4,64,128,256] f32 = 32 MiB in / 32 MiB out dense.

Only cells with n < set_size contribute (the rest of the output is
zero), so the host packs exactly the ACTIVE f-rows (an f-row = 256
contiguous floats for one (b,t,n)) into a contiguous stream, cut into C
uniform [128, W] tiles. Every DMA is therefore a full-width, fully
contiguous, dense transfer — max SDMA efficiency — and the device
program depends only on C, not on the set_size values.

To keep one SPMD program across the 8 cores, rows are sorted by
set_size (descending) per core and the per-rank envelope
sspat[r] = max_i sorted_ss_i[r] defines a shared stream layout; each
core's mask (a [128, C*W/F] f32 tile, one value per f-row, with the
1.1 scale baked in) zeroes its own inactive/padded cells, so the
result is exact. The host scatters the device stream back into the
zero-initialised dense output (device writes every packed byte; no
reliance on output buffer pre-zeroing).

Loads issue on the SP HWDGE ring, stores on the ACT ring (a store
stalled on its mul must not head-block loads); the per-tile mask
multiply alternates between DVE and the otherwise-idle GpSimd so
neither engine's serial chain approaches the DMA span.
"""

import numpy as np

from contextlib import ExitStack

import concourse.bass as bass
import concourse.tile as tile
from concourse import bacc, mybir
from concourse import bass_utils

B, T, N, F = 32, 64, 128, 256
SCALE = 1.1  # ALPHA + BETA
N_CORES = 8
BPC = B // N_CORES            # batches per core = 4
BT = BPC * T                  # 256 flattened (b,t) rows per core
W = 4096                      # stream elems per partition per tile (16 KiB)
WF = W // F                   # f-rows per partition per tile
TILE_ROWS = 128 * WF          # f-rows per tile
BUFS = 6
ALT_RINGS = False             # alternate load/store between SP and ACT rings

_CACHE = {}


def _build(C, W2):
    """Build + compile the SPMD program: C full [128, W] stream tiles plus
    (if W2 > 0) one narrowed [128, W2] tail tile."""
    nc = bacc.Bacc(
        "TRN2",
        target_bir_lowering=False,
        debug=False,
        enable_asserts=False,
        num_devices=N_CORES,
    )
    widths = [W] * C + ([W2] if W2 else [])
    wf_tot = sum(w // F for w in widths)
    total = sum(widths)
    # 1-D stream; each tile is a contiguous [128, w] block (partition-major)
    x = nc.dram_tensor("x", [128 * total], mybir.dt.float32,
                       kind="ExternalInput").ap()
    mask = nc.dram_tensor("mask", [128, wf_tot], mybir.dt.float32,
                          kind="ExternalInput").ap()
    y = nc.dram_tensor("y", [128 * total], mybir.dt.float32,
                       kind="ExternalOutput").ap()

    with tile.TileContext(nc) as tc:
        with ExitStack() as ctx:
            mask_pool = ctx.enter_context(tc.tile_pool(name="maskp", bufs=1))
            pool = ctx.enter_context(tc.tile_pool(name="xp", bufs=BUFS))

            mask_sb = mask_pool.tile([128, wf_tot], mybir.dt.float32,
                                     name="mask_sb")
            nc.scalar.dma_start(mask_sb[:], mask[:])

            off = joff = 0
            for ci, w in enumerate(widths):
                wf = w // F
                xv = x[off:off + 128 * w].rearrange("(p m) -> p m", p=128)
                yv = y[off:off + 128 * w].rearrange("(p m) -> p m", p=128)
                t = pool.tile([128, W], mybir.dt.float32, name="xt")
                swap = ALT_RINGS and ci % 2 == 1
                ld = nc.scalar if swap else nc.sync
                st = nc.sync if swap else nc.scalar
                ld.dma_start(t[:, 0:w], xv)
                t3 = t[:, 0:w].rearrange("p (j f) -> p j f", f=F)
                msl = mask_sb[:, joff:joff + wf]
                nc.vector.tensor_mul(
                    t3, t3, msl.unsqueeze(2).broadcast_to((128, wf, F))
                )
                st.dma_start(yv, t[:, 0:w])
                off += 128 * w
                joff += wf

    nc.compile()
    return nc


def _get_nc(C, W2):
    if (C, W2) not in _CACHE:
        _CACHE[(C, W2)] = _build(C, W2)
    return _CACHE[(C, W2)]


def _host_prep(input_tensor, batch_set_size):
    """Sort rows per core, build the shared stream layout and in_maps."""
    ss_all = np.asarray(batch_set_size).reshape(B, T).astype(np.int64)

    perms = []
    sorted_ss = np.empty((N_CORES, BT), dtype=np.int64)
    for i in range(N_CORES):
        ss = ss_all[i * BPC:(i + 1) * BPC].reshape(BT)
        perm = np.argsort(-ss, kind="stable")
        perms.append(perm)
        sorted_ss[i] = ss[perm]

    sspat = sorted_ss.max(axis=0)                      # envelope, sorted desc
    n_active = int(sspat.sum())                        # active f-rows / core
    if n_active == 0:
        return None, perms, (0, 0), None, 0

    C = n_active // TILE_ROWS
    rem = n_active - C * TILE_ROWS                     # f-rows in tail tile
    W2 = -(-rem // 128) * F if rem else 0              # tail width (elems)
    n_pad = C * TILE_ROWS + 128 * (W2 // F)
    widths = [W] * C + ([W2] if W2 else [])
    wfs = [w // F for w in widths]
    wf_tot = sum(wfs)

    # stream f-row q -> (rank r_arr[q], n = n_arr[q]); same for all cores
    r_arr = np.repeat(np.arange(BT), sspat)
    n_arr = np.concatenate([np.arange(k) for k in sspat if k])
    row_idx = r_arr * N + n_arr                        # into [BT*N] f-rows

    in_maps = []
    for i in range(N_CORES):
        xs = np.asarray(input_tensor[i * BPC:(i + 1) * BPC],
                        dtype=np.float32).reshape(BT, N, F)
        xs_sorted_rows = xs[perms[i]].reshape(BT * N, F)
        x_dev = np.zeros((n_pad, F), dtype=np.float32)
        x_dev[:n_active] = xs_sorted_rows[row_idx]

        m = np.zeros(n_pad, dtype=np.float32)
        m[:n_active] = SCALE * (n_arr < sorted_ss[i][r_arr])
        mdev = np.zeros((128, wf_tot), dtype=np.float32)
        ro = joff = 0
        for wf in wfs:
            mdev[:, joff:joff + wf] = m[ro:ro + 128 * wf].reshape(128, wf)
            ro += 128 * wf
            joff += wf
        in_maps.append({"x": x_dev.reshape(-1), "mask": mdev})
    return in_maps, perms, (C, W2), row_idx, n_active


def kernel(input_tensor, batch_set_size):
    input_tensor = np.asarray(input_tensor, dtype=np.float32)
    in_maps, perms, (C, W2), row_idx, n_active = _host_prep(
        input_tensor, batch_set_size)

    out = np.zeros((B, T, N, F), dtype=np.float32)
    if n_active == 0:
        return out

    nc = _get_nc(C, W2)
    res = bass_utils.run_bass_kernel_spmd(
        nc, in_maps, core_ids=list(range(N_CORES))
    )
    for i in range(N_CORES):
        y_rows = res.results[i]["y"].reshape(-1, F)[:n_active]
        o_sorted = np.zeros((BT * N, F), dtype=np.float32)
        o_sorted[row_idx] = y_rows
        dst = out[i * BPC:(i + 1) * BPC].reshape(BT, N, F)
        dst[perms[i]] = o_sorted.reshape(BT, N, F)
    return out
